# revision 6
# baseline (speedup 1.0000x reference)
"""Trainium2 Bass kernel for a causal multi-head attention layer.

Model: b=2, s=2048, d_model=1024, 16 heads, head_dim=64, pad-index 0.
Sharding over 8 NeuronCores: each core owns 2 heads (128 of the 1024
attention dims) for both batches (head/tensor parallel).  After attention,
an AllToAll redistributes the per-head outputs so each core holds all 1024
attention dims for 1/8 of the sequence positions, where it runs the output
projection locally.  Output rows per core: 256 rows of each batch.
"""

import threading

import numpy as np

B, S, D = 2, 2048, 1024
H, HD = 16, 64
NCORES = 8
LD = D // NCORES          # 128 local attention dims (2 heads)
R = B * S                 # 4096 flattened rows
RC = R // NCORES          # 512 output rows per core
RB = S // NCORES          # 256 rows per batch per core
NKT = S // 128            # 16 key tiles per batch
NCH = D // 128            # 8 contraction chunks of d_model

_cache = {}
_lock = threading.Lock()


def _build_nc():
    import concourse.mybir as mybir
    import concourse.tile as tile
    from concourse import bacc
    from concourse.masks import make_identity
    from contextlib import ExitStack

    f32 = mybir.dt.float32
    bf16 = mybir.dt.bfloat16
    i32 = mybir.dt.int32
    AF = mybir.ActivationFunctionType
    ALU = mybir.AluOpType

    nc = bacc.Bacc(None, target_bir_lowering=False, num_devices=NCORES)

    xT = nc.declare_dram_parameter("xT", [D, R], bf16, isOutput=False)
    wqT = nc.declare_dram_parameter("wqT", [D, LD], bf16, isOutput=False)
    wkT = nc.declare_dram_parameter("wkT", [D, LD], bf16, isOutput=False)
    wvT = nc.declare_dram_parameter("wvT", [D, LD], bf16, isOutput=False)
    woT = nc.declare_dram_parameter("woT", [D, D], bf16, isOutput=False)
    bq = nc.declare_dram_parameter("bq", [LD], f32, isOutput=False)
    bk = nc.declare_dram_parameter("bk", [LD], f32, isOutput=False)
    bv = nc.declare_dram_parameter("bv", [LD], f32, isOutput=False)
    bo = nc.declare_dram_parameter("bo", [D], f32, isOutput=False)
    ids = nc.declare_dram_parameter("ids", [128, B * NKT], i32, isOutput=False)
    out = nc.declare_dram_parameter("out", [RC, D], f32, isOutput=True)

    with ExitStack() as ctx:
        tc = ctx.enter_context(tile.TileContext(nc))
        const = ctx.enter_context(tc.tile_pool(name="const", bufs=1))
        qkp = ctx.enter_context(tc.tile_pool(name="qkp", bufs=2))
        work = ctx.enter_context(tc.tile_pool(name="work", bufs=4))
        est = ctx.enter_context(tc.tile_pool(name="est", bufs=1))
        stg = ctx.enter_context(tc.tile_pool(name="stg", bufs=2))
        spool = ctx.enter_context(tc.tile_pool(name="spool", bufs=2, space="PSUM"))
        opool = ctx.enter_context(tc.tile_pool(name="opool", bufs=4, space="PSUM"))
        dpool = ctx.enter_context(tc.tile_pool(name="dram", bufs=2, space="DRAM"))

        # ---- constants ----
        xT_sb = const.tile([128, NCH, R], bf16)
        nc.sync.dma_start(xT_sb, xT.ap().rearrange("(c p) r -> p c r", p=128))
        woT_sb = const.tile([128, NCH, D], bf16)
        nc.sync.dma_start(woT_sb, woT.ap().rearrange("(c p) n -> p c n", p=128))
        wqT_sb = const.tile([128, NCH, LD], bf16)
        nc.sync.dma_start(wqT_sb, wqT.ap().rearrange("(c p) d -> p c d", p=128))
        wkT_sb = const.tile([128, NCH, LD], bf16)
        nc.sync.dma_start(wkT_sb, wkT.ap().rearrange("(c p) d -> p c d", p=128))
        wvT_sb = const.tile([128, NCH, LD], bf16)
        nc.sync.dma_start(wvT_sb, wvT.ap().rearrange("(c p) d -> p c d", p=128))

        bq_bc = const.tile([128, LD], f32)
        nc.sync.dma_start(bq_bc, bq.ap().partition_broadcast(128))
        bk_bc = const.tile([128, LD], f32)
        nc.sync.dma_start(bk_bc, bk.ap().partition_broadcast(128))
        bv_bc = const.tile([128, LD], f32)
        nc.sync.dma_start(bv_bc, bv.ap().partition_broadcast(128))
        bo_bc = const.tile([128, D], f32)
        nc.sync.dma_start(bo_bc, bo.ap().partition_broadcast(128))

        ids_sb = const.tile([128, B * NKT], i32)
        nc.sync.dma_start(ids_sb, ids.ap())
        padf = const.tile([128, B * NKT], f32)
        nc.vector.tensor_copy(padf, ids_sb)
        nc.vector.tensor_scalar_min(padf, padf, 1.0)

        ident = const.tile([128, 128], bf16)
        make_identity(nc, ident)
        # diagmask[x, y] = 1 if y >= x else 0  (keys on partitions, queries on free)
        diagmask = const.tile([128, 128], bf16)
        nc.gpsimd.memset(diagmask, 1.0)
        nc.gpsimd.affine_select(
            out=diagmask, in_=diagmask, compare_op=ALU.is_ge, fill=0.0,
            base=0, pattern=[[1, 128]], channel_multiplier=-1,
        )

        a2a_outs = []
        for b in range(B):
            # ---- projections for batch b ----
            # QT/KT: [128 dims(2 heads), 2048 rows]; v_aug: [128 keys, head, kt, 65]
            qt_sb = qkp.tile([128, S], bf16, name=f"qt{b}", tag="qt")
            kt_sb = qkp.tile([128, S], bf16, name=f"kt{b}", tag="kt")
            vaug = qkp.tile([128, 2, NKT, HD + 1], bf16, name=f"vaug{b}", tag="vaug")
            for m in range(NKT):
                rsl = slice(b * S + m * 128, b * S + (m + 1) * 128)
                pq = opool.tile([128, LD], f32, name="pq", tag="o")
                pk = opool.tile([128, LD], f32, name="pk", tag="o")
                pv = opool.tile([128, LD], f32, name="pv", tag="o")
                for c in range(NCH):
                    st = c == 0
                    sp = c == NCH - 1
                    lhs = xT_sb[:, c, rsl]
                    nc.tensor.matmul(pq, lhs, wqT_sb[:, c, :], start=st, stop=sp)
                    nc.tensor.matmul(pk, lhs, wkT_sb[:, c, :], start=st, stop=sp)
                    nc.tensor.matmul(pv, lhs, wvT_sb[:, c, :], start=st, stop=sp)
                # Q: add bias, transpose into qt_sb
                tq = work.tile([128, LD], bf16, name="tq", tag="tq")
                nc.vector.tensor_add(tq, pq, bq_bc)
                ptq = spool.tile([128, 128], bf16, name="ptq", tag="s")
                nc.tensor.transpose(ptq, tq, ident)
                nc.vector.tensor_copy(qt_sb[:, m * 128:(m + 1) * 128], ptq)
                # K
                tk = work.tile([128, LD], bf16, name="tk", tag="tq")
                nc.vector.tensor_add(tk, pk, bk_bc)
                ptk = spool.tile([128, 128], bf16, name="ptk", tag="s")
                nc.tensor.transpose(ptk, tk, ident)
                nc.vector.tensor_copy(kt_sb[:, m * 128:(m + 1) * 128], ptk)
                # V: bias, pad-zero rows, ones column (also pad-zeroed)
                tv = work.tile([128, LD], f32, name="tv", tag="tv")
                nc.vector.tensor_add(tv, pv, bv_bc)
                pcol = padf[:, b * NKT + m:b * NKT + m + 1]
                for h in range(2):
                    nc.vector.tensor_scalar_mul(
                        vaug[:, h, m, 0:HD], tv[:, h * HD:(h + 1) * HD], pcol)
                    nc.vector.tensor_copy(vaug[:, h, m, HD:HD + 1], pcol)

            # ---- attention for batch b, heads h=0,1 (local) ----
            stage = stg.tile([128, S], bf16, name=f"stage{b}", tag="stage")
            for h in range(2):
                hsl = slice(h * HD, (h + 1) * HD)
                ests = []
                for kt in range(NKT):
                    q0 = kt * 128          # first visible query
                    w = S - q0             # width of this kt row
                    e = est.tile([128, w], bf16, name=f"e{kt}", tag=f"e{kt}")
                    ests.append(e)
                    # scores in <=1024-wide chunks, exp each chunk
                    off = 0
                    while off < w:
                        cw = min(1024, w - off)
                        ps = spool.tile([128, 1024], f32, name="ps", tag="s")
                        o2 = 0
                        while o2 < cw:
                            mw = min(512, cw - o2)
                            nc.tensor.matmul(
                                ps[:, o2:o2 + mw],
                                kt_sb[hsl, kt * 128:(kt + 1) * 128],
                                qt_sb[hsl, q0 + off + o2:q0 + off + o2 + mw],
                                start=True, stop=True)
                            o2 += mw
                        nc.scalar.activation(
                            e[:, off:off + cw], ps[:, 0:cw], AF.Exp, scale=0.125)
                        off += cw
                    # causal mask on the diagonal 128 columns
                    nc.vector.tensor_mul(e[:, 0:128], e[:, 0:128], diagmask)
                    # PV for query tile m=kt (all needed expst rows now exist)
                    m = kt
                    po = opool.tile([128, HD + 1], f32, name="po", tag="o")
                    for k2 in range(m + 1):
                        nc.tensor.matmul(
                            po,
                            ests[k2][:, (m - k2) * 128:(m - k2) * 128 + 128],
                            vaug[:, h, k2, :],
                            start=(k2 == 0), stop=(k2 == m))
                    rec = work.tile([128, 1], f32, name="rec", tag="rec")
                    nc.vector.reciprocal(rec, po[:, HD:HD + 1])
                    at = work.tile([128, HD], bf16, name="at", tag="at")
                    nc.vector.tensor_scalar_mul(at, po[:, 0:HD], rec)
                    pt = spool.tile([128, 128], bf16, name="pt", tag="s")
                    nc.tensor.transpose(pt[0:HD, :], at, ident)
                    nc.vector.tensor_copy(
                        stage[hsl, m * 128:(m + 1) * 128], pt[0:HD, :])

            # ---- AllToAll for batch b ----
            a2a_in = dpool.tile([NCORES * 128, RB], bf16, name=f"a2ai{b}", tag="a2ai")
            nc.sync.dma_start(
                a2a_in.rearrange("(j p) r -> p j r", p=128),
                stage.rearrange("p (j r) -> p j r", j=NCORES))
            a2a_out = dpool.tile([NCORES * 128, RB], bf16, name=f"a2ao{b}",
                                 tag="a2ao")
            nc.gpsimd.collective_compute(
                "AllToAll", ALU.bypass,
                replica_groups=[list(range(NCORES))],
                ins=[a2a_in.opt()], outs=[a2a_out.opt()])
            a2a_outs.append(a2a_out)

        # ---- output projection (my RB rows of each batch) ----
        for b in range(B):
            a2a_sb = stg.tile([128, NCORES, RB], bf16, name=f"a2as{b}", tag="a2as")
            nc.sync.dma_start(
                a2a_sb, a2a_outs[b].rearrange("(j p) r -> p j r", p=128))
            for m in range(RB // 128):
                for n in range(D // 512):
                    pout = opool.tile([128, 512], f32, name="pout", tag="o")
                    for c in range(NCH):
                        nc.tensor.matmul(
                            pout,
                            a2a_sb[:, c, m * 128:(m + 1) * 128],
                            woT_sb[:, c, n * 512:(n + 1) * 512],
                            start=(c == 0), stop=(c == NCH - 1))
                    ot = work.tile([128, 512], f32, name="ot", tag="ot")
                    nc.vector.tensor_add(ot, pout, bo_bc[:, n * 512:(n + 1) * 512])
                    nc.sync.dma_start(
                        out.ap()[b * RB + m * 128:b * RB + (m + 1) * 128,
                                 n * 512:(n + 1) * 512], ot)

    nc.finalize()
    return nc


def _get_nc():
    with _lock:
        if "nc" not in _cache:
            _cache["nc"] = _build_nc()
        return _cache["nc"]


def _shard_inputs(x, input_ids, Wq, bq, Wk, bk, Wv, bv, Wo, bo):
    import ml_dtypes
    bf16 = ml_dtypes.bfloat16

    x = np.asarray(x, dtype=np.float32)
    xT = np.ascontiguousarray(x.reshape(R, D).T).astype(bf16)
    woT = np.ascontiguousarray(np.asarray(Wo, dtype=np.float32).T).astype(bf16)
    bo_f = np.asarray(bo, dtype=np.float32)
    ids = np.asarray(input_ids).astype(np.int32)
    # ids_r[p, b*NKT + t] = input_ids[b, t*128 + p]
    ids_r = np.ascontiguousarray(ids.reshape(B, NKT, 128).transpose(2, 0, 1)
                                 .reshape(128, B * NKT))
    Wq = np.asarray(Wq, dtype=np.float32)
    Wk = np.asarray(Wk, dtype=np.float32)
    Wv = np.asarray(Wv, dtype=np.float32)
    bq = np.asarray(bq, dtype=np.float32)
    bk = np.asarray(bk, dtype=np.float32)
    bv = np.asarray(bv, dtype=np.float32)

    in_maps = []
    for c in range(NCORES):
        sl = slice(c * LD, (c + 1) * LD)
        in_maps.append({
            "xT": xT,
            "wqT": np.ascontiguousarray(Wq[sl].T).astype(bf16),
            "wkT": np.ascontiguousarray(Wk[sl].T).astype(bf16),
            "wvT": np.ascontiguousarray(Wv[sl].T).astype(bf16),
            "woT": woT,
            "bq": bq[sl].copy(),
            "bk": bk[sl].copy(),
            "bv": bv[sl].copy(),
            "bo": bo_f,
            "ids": ids_r,
        })
    return in_maps


def run(trace=False, **inputs):
    """Run the kernel; returns (output, BassKernelResults)."""
    from concourse.bass_utils import run_bass_kernel_spmd

    nc = _get_nc()
    in_maps = _shard_inputs(**inputs)
    res = run_bass_kernel_spmd(nc, in_maps, core_ids=list(range(NCORES)),
                               trace=trace)
    full = np.empty((B, S, D), dtype=np.float32)
    for c in range(NCORES):
        o = np.asarray(res.results[c]["out"], dtype=np.float32)
        for b in range(B):
            full[b, c * RB:(c + 1) * RB, :] = o[b * RB:(b + 1) * RB, :]
    return full, res


def kernel(**inputs) -> np.ndarray:
    full, _ = run(trace=False, **inputs)
    return full
